# revision 9
# baseline (speedup 1.0000x reference)
"""AdaptiveGridMerger Trainium2 kernel.

Math: reference scatters x[b,c,:] into a flat 8x8 grid with bilinear
(4-corner) weights from positions[b,c,:], then matmuls grid_weights.
Equivalent form used here: out[b] = GW @ (S[b] @ x[b]) where
S[b] in R[64,306] has the 4 corner weights of channel c in column c.
S[b].T (layout [c, g]) is built on-device with iota/is_equal one-hot
compares, then both contractions run on the TensorEngine.

Sharding: data-parallel over batch, 2 batches per core, grid_weights
replicated (pre-transposed on host to [64, 270] for the lhsT layout).
"""

import numpy as np

import concourse.bass as bass
import concourse.bacc as bacc
import concourse.mybir as mybir
from concourse import tile
from concourse.bass_utils import run_bass_kernel_spmd

B, C, T = 16, 306, 4096
M, G, GS = 270, 64, 8
N_CORES = 8
BL = B // N_CORES  # batches per core

C_CHUNKS = [(0, 128), (128, 128), (256, 50)]
M_CHUNKS = [(0, 128), (128, 128), (256, 14)]
T_DMA = 2048
T_PS = 512

# float32r (= tf32, 10-bit mantissa) streams 1 col/cycle on the PE vs 4 for
# float32. Matmul inputs are declared float32r end-to-end; x and gw_t are
# pre-rounded to tf32 on the host so the DMA chain carries rounded values.
MM_DTYPE = mybir.dt.float32r

FP32 = mybir.dt.float32
OP = mybir.AluOpType


def _round_tf32(a):
    if MM_DTYPE == FP32:
        return np.ascontiguousarray(a.astype(np.float32))
    u = np.ascontiguousarray(a.astype(np.float32)).view(np.uint32)
    u = u + 0x0FFF + ((u >> 13) & 1)
    u &= np.uint32(0xFFFFE000)
    return u.view(np.float32)


def build_nc():
    nc = bacc.Bacc()
    x_ext = nc.declare_dram_parameter("x", [BL, C, T], MM_DTYPE, isOutput=False)
    pos_ext = nc.declare_dram_parameter("positions", [BL, C, 2], FP32, isOutput=False)
    gwt_ext = nc.declare_dram_parameter("gw_t", [G, M], MM_DTYPE, isOutput=False)
    out_ext = nc.declare_dram_parameter("out", [BL, M, T], FP32, isOutput=True)

    with tile.TileContext(nc) as tc:
        with (
            tc.tile_pool(name="const", bufs=1) as constp,
            tc.tile_pool(name="stp", bufs=1) as stp,
            tc.tile_pool(name="scr", bufs=6) as scr,
            tc.tile_pool(name="xp", bufs=2) as xp,
            tc.tile_pool(name="op", bufs=2) as outp,
            tc.tile_pool(name="gvp", bufs=3) as gvp,
            tc.tile_pool(name="ps_gv", bufs=2, space=bass.MemorySpace.PSUM) as ps_gv,
            tc.tile_pool(name="ps_out", bufs=4, space=bass.MemorySpace.PSUM) as ps_out,
        ):
            gw_t = constp.tile([G, M], MM_DTYPE, tag="gw_t")
            nc.sync.dma_start(out=gw_t[:], in_=gwt_ext[:])

            # iota row [j - k for j in 0..63]; compare against idx_ll gives the
            # one-hot for corner offset k (k in {0,1,8,9} = {ll,lh,hl,hh}).
            # gpsimd generates the iotas; DVE copies absorb the cross-engine
            # wait once so the hot per-chunk DVE ops have same-engine deps only
            # (TensorScalar has too few sync-wait slots for multi-engine deps).
            iotas = []
            for k in (0, 1, GS, GS + 1):
                it_g = constp.tile([128, G], FP32, tag=f"iotag{k}", name=f"iotag{k}")
                nc.gpsimd.iota(
                    it_g[:],
                    pattern=[[1, G]],
                    base=-k,
                    channel_multiplier=0,
                    allow_small_or_imprecise_dtypes=True,
                )
                it = constp.tile([128, G], FP32, tag=f"iota{k}", name=f"iota{k}")
                nc.vector.tensor_copy(it[:], it_g[:])
                iotas.append(it)

            # Build ST[c, g] = sum_corners w_corner[c] * (g == idx_corner[c])
            st_tiles = {}
            for b in range(BL):
                for ci, (c0, cn) in enumerate(C_CHUNKS):
                    pos_t = scr.tile([128, 2], FP32, tag="pos")
                    nc.sync.dma_start(out=pos_t[:cn, :], in_=pos_ext[b, c0 : c0 + cn, :])
                    # grid_pos = (pos + 1) * 4, exact vs reference's *8/2
                    gp = scr.tile([128, 2], FP32, tag="gp")
                    nc.vector.tensor_scalar(
                        gp[:cn], pos_t[:cn], 1.0, GS / 2.0, OP.add, OP.mult
                    )
                    # floor(): int cast then subtract 1 where the cast rounded up
                    ilow = scr.tile([128, 2], mybir.dt.int32, tag="ilow")
                    nc.vector.tensor_copy(ilow[:cn], gp[:cn])
                    flow = scr.tile([128, 2], FP32, tag="flow")
                    nc.vector.tensor_copy(flow[:cn], ilow[:cn])
                    mask = scr.tile([128, 2], FP32, tag="mask")
                    nc.vector.tensor_tensor(mask[:cn], flow[:cn], gp[:cn], OP.is_gt)
                    low = scr.tile([128, 2], FP32, tag="low")
                    nc.vector.tensor_tensor(low[:cn], flow[:cn], mask[:cn], OP.subtract)

                    whi = scr.tile([128, 2], FP32, tag="whi")
                    nc.vector.tensor_tensor(whi[:cn], gp[:cn], low[:cn], OP.subtract)
                    wlo = scr.tile([128, 2], FP32, tag="wlo")
                    nc.vector.tensor_scalar(
                        wlo[:cn], whi[:cn], -1.0, 1.0, OP.mult, OP.add
                    )
                    idx = scr.tile([128, 1], FP32, tag="idx")
                    nc.vector.scalar_tensor_tensor(
                        idx[:cn],
                        in0=low[:cn, 0:1],
                        scalar=float(GS),
                        in1=low[:cn, 1:2],
                        op0=OP.mult,
                        op1=OP.add,
                    )

                    st = stp.tile([128, G], MM_DTYPE, tag=f"st{b}_{ci}")
                    corners = ((wlo, wlo), (wlo, whi), (whi, wlo), (whi, whi))
                    for k, (wa, wb) in enumerate(corners):
                        w = scr.tile([128, 1], FP32, tag=f"w{k}")
                        nc.vector.tensor_tensor(
                            w[:cn], wa[:cn, 0:1], wb[:cn, 1:2], OP.mult
                        )
                        if k == 0:
                            nc.vector.tensor_scalar(
                                st[:cn], iotas[k][:cn], idx[:cn], w[:cn],
                                OP.is_equal, OP.mult,
                            )
                        else:
                            term = scr.tile([128, G], FP32, tag="term")
                            nc.vector.tensor_scalar(
                                term[:cn], iotas[k][:cn], idx[:cn], w[:cn],
                                OP.is_equal, OP.mult,
                            )
                            nc.vector.tensor_tensor(
                                st[:cn], st[:cn], term[:cn], OP.add
                            )
                    st_tiles[(b, ci)] = st

            # Warm-up matmuls: force the PE to observe the DVE semaphore (all
            # ST tiles done) and gw_t's DMA queue once, so steady-state
            # matmuls carry at most ONE sync wait (codegen limit on LW).
            with tc.tile_pool(name="ps_warm", bufs=1, space=bass.MemorySpace.PSUM) as ps_warm:
                warm = ps_warm.tile([128, G], FP32, tag="warm")
                for b in range(BL):
                    for ci, (c0, cn) in enumerate(C_CHUNKS):
                        st = st_tiles[(b, ci)]
                        nc.tensor.matmul(
                            warm[:G], st[:cn], st[:cn, :G], start=True, stop=True
                        )
                nc.tensor.matmul(
                    warm[:, :G], gw_t[:, :128], gw_t[:, :G], start=True, stop=True
                )

            # Main loop: gv = ST.T @ x (per t-chunk), out = GW @ gv
            for b in range(BL):
                for tt in range(T // T_DMA):
                    t0 = tt * T_DMA
                    xts = []
                    for ci, (c0, cn) in enumerate(C_CHUNKS):
                        xt = xp.tile([128, T_DMA], MM_DTYPE, tag=f"x{ci}")
                        nc.sync.dma_start(
                            out=xt[:cn], in_=x_ext[b, c0 : c0 + cn, t0 : t0 + T_DMA]
                        )
                        xts.append(xt)
                    outs = []
                    for mi, (m0, mn) in enumerate(M_CHUNKS):
                        outs.append(
                            outp.tile([128, T_DMA], FP32, tag=f"o{mi}", name=f"o{mi}")
                        )

                    for ps in range(T_DMA // T_PS):
                        f0 = ps * T_PS
                        gv_ps = ps_gv.tile([G, T_PS], FP32, tag="gv")
                        for ci, (c0, cn) in enumerate(C_CHUNKS):
                            nc.tensor.matmul(
                                gv_ps[:],
                                st_tiles[(b, ci)][:cn],
                                xts[ci][:cn, f0 : f0 + T_PS],
                                start=(ci == 0),
                                stop=(ci == len(C_CHUNKS) - 1),
                            )
                        gv_sb = gvp.tile([G, T_PS], MM_DTYPE, tag="gv_sb")
                        nc.vector.tensor_copy(gv_sb[:], gv_ps[:])
                        for mi, (m0, mn) in enumerate(M_CHUNKS):
                            o_ps = ps_out.tile([128, T_PS], FP32, tag="o_ps")
                            nc.tensor.matmul(
                                o_ps[:mn],
                                gw_t[:, m0 : m0 + mn],
                                gv_sb[:],
                                start=True,
                                stop=True,
                            )
                            nc.vector.tensor_copy(
                                outs[mi][:mn, f0 : f0 + T_PS], o_ps[:mn]
                            )
                    for mi, (m0, mn) in enumerate(M_CHUNKS):
                        nc.sync.dma_start(
                            out=out_ext[b, m0 : m0 + mn, t0 : t0 + T_DMA],
                            in_=outs[mi][:mn],
                        )
    nc.compile()
    return nc


def make_in_maps(x, positions, grid_weights):
    gw_t = _round_tf32(grid_weights.T)
    in_maps = []
    for i in range(N_CORES):
        sl = slice(i * BL, (i + 1) * BL)
        in_maps.append(
            {
                "x": _round_tf32(x[sl]),
                "positions": np.ascontiguousarray(positions[sl]),
                "gw_t": gw_t,
            }
        )
    return in_maps


_NC_CACHE = None


def kernel(x, positions, grid_weights):
    global _NC_CACHE
    if _NC_CACHE is None:
        _NC_CACHE = build_nc()
    nc = _NC_CACHE
    in_maps = make_in_maps(x, positions, grid_weights)
    res = run_bass_kernel_spmd(nc, in_maps, core_ids=list(range(N_CORES)))
    out = np.concatenate([r["out"] for r in res.results], axis=0)
    return out.astype(np.float32)


if __name__ == "__main__":
    xs = np.random.randn(B, C, T).astype(np.float32)
    ps = np.random.uniform(-1, 0.74, (B, C, 2)).astype(np.float32)
    gw = np.random.randn(M, G).astype(np.float32)
    out = kernel(xs, ps, gw)
    print(out.shape, out.dtype)
